# revision 2
# baseline (speedup 1.0000x reference)
"""DeepSeek-V2-style MoE kernel for 8 Trainium2 NeuronCores.

Strategy (expert-parallel, dense):
- 16 experts, 8 cores -> 2 experts per core. Each core computes its two
  experts' SwiGLU MLPs densely over all 1024 tokens (bf16 matmuls, fp32
  accumulate), weighted by on-device routing weights.
- The shared expert is sharded over its intermediate dim (256 of 2048 per
  core); its per-core partial seeds the routed combine, so one
  ReduceScatter(add) of the [T, H] partial produces each core's final
  128-token output shard directly.
- The gate (sigmoid + grouped top-k) runs on every core in fp32 so expert
  selection matches the fp32 reference exactly. The expert axis is permuted
  per core so each core's own experts sit at positions 0 and 1.
- All per-core tensors are packed into THREE runtime args (weight blob
  bf16, h^T fp32, gate meta fp32): the PJRT/axon exec path charges ~60us
  per input arg per execution, so arg count dominates the measured time.
  The bf16 copy of h^T is derived on device from the fp32 arg.
"""

import os
import sys

import numpy as np
import ml_dtypes

for _p in ("/opt/trn_rl_repo", os.path.expanduser("~/.axon_site/_ro/trn_rl_repo")):
    if os.path.isdir(_p) and _p not in sys.path:
        sys.path.append(_p)

import concourse.bass as bass
import concourse.mybir as mybir
import concourse.tile as tile
from concourse.bass_utils import run_bass_kernel_spmd

# problem sizes (fixed)
T, H, E, I, SI = 1024, 1024, 16, 704, 2048
P = 128
NCORES = 8
KT = H // P            # 8 contraction tiles over H
IT = 6                 # ceil(704/128) I tiles; last is 64 rows (wd zero-padded)
IPAD = IT * P          # 768
SIC = SI // NCORES     # 256: shared-expert intermediate slice per core
SICT = SIC // P        # 2
NB = 2                 # token blocks
BLK = T // NB          # 512
MSUB = BLK // P        # 4 token subtiles per block
BIG = 1.0e6
OFF = 10.0             # offset making all valid masked scores positive

# blob column layout (bf16, [P, BCOLS]); order = DMA issue order so compute
# can chase the loads: shared expert first, then e0 up, e0 down, e1 up, e1 down
C_SWG = 0
C_SWU = C_SWG + KT * SIC          # 2048
C_SWD = C_SWU + KT * SIC          # 4096
C_WG0 = C_SWD + SICT * H          # 6144
C_WU0 = C_WG0 + KT * I            # 11776
C_WD0 = C_WU0 + KT * I            # 17408
C_WG1 = C_WD0 + IT * H            # 23552
C_WU1 = C_WG1 + KT * I            # 29184
C_WD1 = C_WU1 + KT * I            # 34816
BCOLS = C_WD1 + IT * H            # 40960

F32 = mybir.dt.float32
BF16 = mybir.dt.bfloat16
ALU = mybir.AluOpType
ACTF = mybir.ActivationFunctionType

_BUILD_CACHE = {}


def _split_sync_waits(nc):
    """This walrus build allows one sync wait per instruction; move extra
    waits onto same-engine pure-wait carriers placed immediately before."""
    n_split = 0
    for f in nc.m.functions:
        for bb in f.blocks:
            out = []
            for ins in bb.instructions:
                si = ins.sync_info
                if si is not None and si.on_wait and len(si.on_wait) > 1:
                    waits = list(si.on_wait)
                    head, tail = waits[:-1], waits[-1:]
                    for i, w in enumerate(head):
                        carrier = mybir.InstEventSemaphore(
                            name=f"{ins.name}-ws{i}",
                            engine=ins.engine,
                            ins=[],
                            outs=[],
                            sync_info=mybir.SyncInfo(on_wait=[w], on_update=[]),
                        )
                        nc.register_instruction(carrier, overwrite=True)
                        out.append(carrier)
                    ins.sync_info = mybir.SyncInfo(on_wait=tail,
                                                   on_update=si.on_update)
                    n_split += 1
                out.append(ins)
            bb.instructions[:] = out
    return nc


def _build(with_collective=True, routed_reps=1, shared_reps=1, coll_reps=1):
    nc = bass.Bass(num_devices=NCORES)

    # ---- three packed runtime parameters (per-core contents host-side) ----
    blob = nc.declare_dram_parameter("blob", [P, BCOLS], BF16, isOutput=False)
    hx = nc.declare_dram_parameter("hx", [P, KT * T], F32, isOutput=False)
    gmeta = nc.declare_dram_parameter("gmeta", [P, 144], F32, isOutput=False)
    out = nc.declare_dram_parameter("out", [P, H], F32, isOutput=True)

    with tile.TileContext(nc) as tc:
        with (
            tc.tile_pool(name="const", bufs=1) as const,
            tc.tile_pool(name="wpool", bufs=1) as wpool,
            tc.tile_pool(name="apool", bufs=2) as apool,
            tc.tile_pool(name="stmp", bufs=2) as stmp,
            tc.tile_pool(name="part", bufs=2) as part,
            tc.tile_pool(name="rpool", bufs=1) as rpool,
            tc.tile_pool(name="pgu", bufs=4, space="PSUM") as pgu,
            tc.tile_pool(name="py", bufs=4, space="PSUM") as py,
            tc.tile_pool(name="dram", bufs=1, space="DRAM") as dram,
        ):
            # ------------- loads: big blob in 5 chunks across queues -------
            bsb = wpool.tile([P, BCOLS], BF16, name="bsb")
            nc.scalar.dma_start(out=bsb[:, C_SWG:C_WG0], in_=blob[:, C_SWG:C_WG0])
            nc.sync.dma_start(out=bsb[:, C_WG0:C_WD0], in_=blob[:, C_WG0:C_WD0])
            nc.gpsimd.dma_start(out=bsb[:, C_WD0:C_WG1], in_=blob[:, C_WD0:C_WG1])
            nc.sync.dma_start(out=bsb[:, C_WG1:C_WD1], in_=blob[:, C_WG1:C_WD1])
            nc.gpsimd.dma_start(out=bsb[:, C_WD1:BCOLS], in_=blob[:, C_WD1:BCOLS])

            swg_sb = bsb[:, C_SWG:C_SWU].rearrange("p (k c) -> p k c", k=KT)
            swu_sb = bsb[:, C_SWU:C_SWD].rearrange("p (k c) -> p k c", k=KT)
            swd_sb = bsb[:, C_SWD:C_WG0].rearrange("p (i h) -> p i h", i=SICT)
            wg_sb = [bsb[:, C_WG0:C_WU0].rearrange("p (k i) -> p k i", k=KT),
                     bsb[:, C_WG1:C_WU1].rearrange("p (k i) -> p k i", k=KT)]
            wu_sb = [bsb[:, C_WU0:C_WD0].rearrange("p (k i) -> p k i", k=KT),
                     bsb[:, C_WU1:C_WD1].rearrange("p (k i) -> p k i", k=KT)]
            wd_sb = [bsb[:, C_WD0:C_WG1].rearrange("p (i h) -> p i h", i=IT),
                     bsb[:, C_WD1:BCOLS].rearrange("p (i h) -> p i h", i=IT)]

            hx_sb = const.tile([P, KT, T], F32, name="hx_sb")
            nc.scalar.dma_start(out=hx_sb[:].rearrange("p k t -> p (k t)"),
                                in_=hx[:])
            gm_sb = const.tile([P, 144], F32, name="gm_sb")
            nc.sync.dma_start(out=gm_sb[:], in_=gmeta[:])
            gw_sb = gm_sb[:, 0:128].rearrange("p (k e) -> p k e", k=KT)
            bias16 = gm_sb[:, 128:144]

            # bf16 copy of h^T for the expert matmuls
            ht_sb = const.tile([P, KT, T], BF16, name="ht_sb")
            for k in range(KT):
                nc.vector.tensor_copy(ht_sb[:, k, :], hx_sb[:, k, :])

            scores = rpool.tile([P, P], F32, name="scores")

            # ------------- shared expert (intermediate slice, all tokens) --
            As = const.tile([P, SICT, T], BF16, name="As_sh")
            ys = const.tile([P, NB * MSUB, 2, 512], BF16, name="ys")
            for rep_s in range(shared_reps):
                for si in range(SICT):
                    for b in range(NB):
                        tsl = slice(b * BLK, (b + 1) * BLK)
                        pGs = pgu.tile([P, 512], F32, name="pgs", tag="pgu")
                        pUs = pgu.tile([P, 512], F32, name="pus", tag="pgu")
                        for k in range(KT):
                            nc.tensor.matmul(
                                pGs[:, :], lhsT=swg_sb[:, k, si * P:(si + 1) * P],
                                rhs=ht_sb[:, k, tsl],
                                start=(k == 0), stop=(k == KT - 1))
                        for k in range(KT):
                            nc.tensor.matmul(
                                pUs[:, :], lhsT=swu_sb[:, k, si * P:(si + 1) * P],
                                rhs=ht_sb[:, k, tsl],
                                start=(k == 0), stop=(k == KT - 1))
                        sts = stmp.tile([P, BLK], F32, name="st", tag="st")
                        nc.scalar.activation(sts[:, :], pGs[:, :], ACTF.Silu)
                        nc.vector.tensor_tensor(As[:, si, tsl], sts[:, :],
                                                pUs[:, :], op=ALU.mult)
                if rep_s == 0:
                    for tt in range(8):
                        pg = pgu.tile([P, 512], F32, name="pgate", tag="pgu")
                        for k in range(KT):
                            nc.tensor.matmul(pg[:, :E],
                                             lhsT=hx_sb[:, k, tt * P:(tt + 1) * P],
                                             rhs=gw_sb[:, k, :],
                                             start=(k == 0), stop=(k == KT - 1))
                        nc.scalar.activation(scores[:, tt * E:(tt + 1) * E],
                                             pg[:, :E], ACTF.Sigmoid)
                for mg in range(NB * MSUB):
                    for n in range(2):
                        pYs = py.tile([P, 512], F32, name="pys", tag="py")
                        for si in range(SICT):
                            nc.tensor.matmul(
                                pYs[:, :],
                                lhsT=As[:, si, mg * P:(mg + 1) * P],
                                rhs=swd_sb[:, si, n * 512:(n + 1) * 512],
                                start=(si == 0), stop=(si == SICT - 1))
                        nc.scalar.activation(ys[:, mg, n, :], pYs[:, :],
                                             ACTF.Copy)

            # ------------- routing -------------
            sfc = rpool.tile([P, P], F32, name="sfc")
            biasb = bias16.rearrange("p (o e) -> p o e", o=1) \
                .broadcast_to([P, 8, E])
            nc.vector.tensor_tensor(
                sfc[:].rearrange("p (t e) -> p t e", t=8),
                scores[:].rearrange("p (t e) -> p t e", t=8), biasb, op=ALU.add)
            v4 = sfc[:].rearrange("p (t g e) -> p t g e", t=8, g=4, e=4)

            def t32(nm):
                return rpool.tile([P, 32], F32, name=nm)

            def v32(t):
                return t[:].rearrange("p (t g) -> p t g", t=8)

            a_, b_, c_, d_ = (v4[:, :, :, j] for j in range(4))
            m1, n1, m2, n2 = t32("m1"), t32("n1"), t32("m2"), t32("n2")
            top1, t3, t4, sec, gs = (t32(x) for x in
                                     ("top1", "t3", "t4", "sec", "gs"))
            nc.vector.tensor_tensor(v32(m1), a_, b_, op=ALU.max)
            nc.vector.tensor_tensor(v32(n1), a_, b_, op=ALU.min)
            nc.vector.tensor_tensor(v32(m2), c_, d_, op=ALU.max)
            nc.vector.tensor_tensor(v32(n2), c_, d_, op=ALU.min)
            nc.vector.tensor_tensor(top1[:], m1[:], m2[:], op=ALU.max)
            nc.vector.tensor_tensor(t3[:], m1[:], m2[:], op=ALU.min)
            nc.vector.tensor_tensor(t4[:], n1[:], n2[:], op=ALU.max)
            nc.vector.tensor_tensor(sec[:], t3[:], t4[:], op=ALU.max)
            nc.vector.tensor_tensor(gs[:], top1[:], sec[:], op=ALU.add)

            gv = gs[:].rearrange("p (t g) -> p t g", t=8)

            def t8(nm):
                return rpool.tile([P, 8], F32, name=nm)

            u1, l1, u2, l2, q1, q2, thr = (t8(x) for x in
                                           ("u1", "l1", "u2", "l2", "q1", "q2",
                                            "thr"))
            x0, x1, x2, x3 = (gv[:, :, j] for j in range(4))
            nc.vector.tensor_tensor(u1[:], x0, x1, op=ALU.max)
            nc.vector.tensor_tensor(l1[:], x0, x1, op=ALU.min)
            nc.vector.tensor_tensor(u2[:], x2, x3, op=ALU.max)
            nc.vector.tensor_tensor(l2[:], x2, x3, op=ALU.min)
            nc.vector.tensor_tensor(q1[:], u1[:], u2[:], op=ALU.min)
            nc.vector.tensor_tensor(q2[:], l1[:], l2[:], op=ALU.max)
            nc.vector.tensor_tensor(thr[:], q1[:], q2[:], op=ALU.max)

            pen = t32("pen")
            thrb = thr[:].rearrange("p (t o) -> p t o", o=1) \
                .broadcast_to([P, 8, 4])
            nc.vector.tensor_tensor(v32(pen), gv, thrb, op=ALU.is_lt)
            nc.vector.tensor_scalar_mul(pen[:], pen[:], BIG)

            masked = rpool.tile([P, P], F32, name="masked")
            mv4 = masked[:].rearrange("p (t g e) -> p t g e", t=8, g=4, e=4)
            penb = pen[:].rearrange("p (t g o) -> p t g o", t=8, o=1) \
                .broadcast_to([P, 8, 4, 4])
            nc.vector.scalar_tensor_tensor(mv4, v4, OFF, penb,
                                           op0=ALU.add, op1=ALU.subtract)

            mv3 = masked[:].rearrange("p (t e) -> p t e", t=8)
            mx = t8("mx")
            lt = rpool.tile([P, P], F32, name="lt")
            lt3 = lt[:].rearrange("p (t e) -> p t e", t=8)
            for _ in range(6):
                nc.vector.tensor_reduce(mx[:], mv3, axis=mybir.AxisListType.X,
                                        op=ALU.max)
                mxb = mx[:].rearrange("p (t o) -> p t o", o=1) \
                    .broadcast_to([P, 8, 16])
                nc.vector.tensor_tensor(lt3, mv3, mxb, op=ALU.is_lt)
                nc.vector.tensor_tensor(masked[:], lt[:], masked[:],
                                        op=ALU.mult)

            sel = rpool.tile([P, P], F32, name="sel")
            nc.vector.tensor_scalar(sel[:], masked[:], 0.0, None,
                                    op0=ALU.is_equal)
            sw = rpool.tile([P, P], F32, name="swt")
            nc.vector.tensor_tensor(sw[:], scores[:], sel[:], op=ALU.mult)
            sums = t8("sums")
            nc.vector.tensor_reduce(sums[:],
                                    sw[:].rearrange("p (t e) -> p t e", t=8),
                                    axis=mybir.AxisListType.X, op=ALU.add)
            rec = t8("rec")
            nc.vector.reciprocal(rec[:], sums[:])
            cw = [rpool.tile([P, 8], F32, name=f"cw{e}") for e in range(2)]
            swv = sw[:].rearrange("p (t e) -> p t e", t=8)
            for e in range(2):
                for tt in range(8):
                    nc.vector.scalar_tensor_tensor(
                        cw[e][:, tt:tt + 1], swv[:, tt, e:e + 1], 2.0,
                        rec[:, tt:tt + 1], op0=ALU.mult, op1=ALU.mult)

            # ------------- DRAM partials & collectives -------------
            partial = [dram.tile([T, 512], F32, name=f"partial{n}")
                       for n in range(2)]
            rs = [dram.tile([P, 512], F32, name=f"rs{n}") for n in range(2)]

            # ------------- routed experts -------------
            for rep, b in [(rep, b) for rep in range(routed_reps)
                           for b in range(NB)]:
                last_rep = rep == routed_reps - 1
                tsl = slice(b * BLK, (b + 1) * BLK)
                A = []
                for e in range(2):
                    At = apool.tile([P, IT, BLK], BF16, name=f"A{e}",
                                    tag=f"A{e}")
                    nc.vector.memset(At[P - 64:, IT - 1, :], 0.0)
                    for i in range(IT):
                        ip = P if i < IT - 1 else I - (IT - 1) * P
                        pG = pgu.tile([P, 512], F32, name="pgu", tag="pgu")
                        pU = pgu.tile([P, 512], F32, name="pgu2", tag="pgu")
                        for k in range(KT):
                            nc.tensor.matmul(
                                pG[:ip, :],
                                lhsT=wg_sb[e][:, k, i * P:i * P + ip],
                                rhs=ht_sb[:, k, tsl],
                                start=(k == 0), stop=(k == KT - 1))
                        for k in range(KT):
                            nc.tensor.matmul(
                                pU[:ip, :],
                                lhsT=wu_sb[e][:, k, i * P:i * P + ip],
                                rhs=ht_sb[:, k, tsl],
                                start=(k == 0), stop=(k == KT - 1))
                        st = stmp.tile([P, BLK], F32, name="st", tag="st")
                        nc.scalar.activation(st[:ip, :], pG[:ip, :], ACTF.Silu)
                        nc.vector.tensor_tensor(At[:ip, i, :], st[:ip, :],
                                                pU[:ip, :], op=ALU.mult)
                    A.append(At)

                for n in range(2):
                    pt = part.tile([P, MSUB, 512], F32, name="pt", tag="pt")
                    for e in range(2):
                        for m in range(MSUB):
                            pY = py.tile([P, 512], F32, name="py", tag="py")
                            for i in range(IT):
                                nc.tensor.matmul(
                                    pY[:, :],
                                    lhsT=A[e][:, i, m * P:(m + 1) * P],
                                    rhs=wd_sb[e][:, i, n * 512:(n + 1) * 512],
                                    start=(i == 0), stop=(i == IT - 1))
                            tt = b * MSUB + m
                            if e == 0:
                                # seed with the shared-expert partial
                                nc.vector.scalar_tensor_tensor(
                                    pt[:, m, :], pY[:, :], cw[0][:, tt:tt + 1],
                                    ys[:, tt, n, :], op0=ALU.mult, op1=ALU.add)
                            else:
                                nc.vector.scalar_tensor_tensor(
                                    pt[:, m, :], pY[:, :], cw[1][:, tt:tt + 1],
                                    pt[:, m, :], op0=ALU.mult, op1=ALU.add)
                    if last_rep:
                        for m in range(MSUB):
                            r0 = b * BLK + m * P
                            nc.sync.dma_start(
                                out=partial[n][r0:r0 + P, :],
                                in_=pt[:, m, :])
                    if last_rep and b == NB - 1 and with_collective:
                        for _cr in range(coll_reps):
                            nc.gpsimd.collective_compute(
                                "ReduceScatter", ALU.add,
                                replica_groups=[list(range(NCORES))],
                                ins=[partial[n][:]], outs=[rs[n][:]])

            # ------------- epilogue -------------
            for n in range(2):
                if with_collective:
                    nc.sync.dma_start(out=out[:, n * 512:(n + 1) * 512],
                                      in_=rs[n][:])
                else:
                    nc.sync.dma_start(out=out[:, n * 512:(n + 1) * 512],
                                      in_=partial[n][0:P, :])

    _split_sync_waits(nc)
    return nc


def _perm_for_core(c):
    g_sel = c >> 1
    rot = 2 * (c & 1)
    perm = [4 * g_sel + ((rot + j) % 4) for j in range(4)]
    for g in range(4):
        if g != g_sel:
            perm.extend(range(4 * g, 4 * g + 4))
    return perm


def _pk(w, k):
    """[k*P, X] -> [P, k, X] partition-major blocking."""
    return np.ascontiguousarray(
        w.reshape(k, P, w.shape[1]).transpose(1, 0, 2))


def prepare_in_maps(h, gate_w, bias, wg, wu, wd, swg, swu, swd):
    bf = ml_dtypes.bfloat16
    h = np.asarray(h, np.float32)
    gate_w = np.asarray(gate_w, np.float32)
    bias = np.asarray(bias, np.float32)

    ht32 = np.ascontiguousarray(h.T)                      # [H, T] f32
    hx = _pk(ht32, KT).reshape(P, KT * T)                 # [P, 8*1024] f32
    gwt = np.ascontiguousarray(gate_w.T)                  # [H, E] f32

    swg32 = np.asarray(swg, np.float32)
    swu32 = np.asarray(swu, np.float32)
    swd32 = np.asarray(swd, np.float32)

    wd_pad = np.zeros((E, IPAD, H), np.float32)
    wd_pad[:, :I, :] = np.asarray(wd, np.float32)

    in_maps = []
    for c in range(NCORES):
        e0, e1 = 2 * c, 2 * c + 1
        perm = _perm_for_core(c)
        csl = slice(c * SIC, (c + 1) * SIC)

        blob = np.empty((P, BCOLS), bf)
        blob[:, C_SWG:C_SWU] = _pk(swg32[:, csl], KT).reshape(P, -1)
        blob[:, C_SWU:C_SWD] = _pk(swu32[:, csl], KT).reshape(P, -1)
        blob[:, C_SWD:C_WG0] = _pk(swd32[csl, :], SICT).reshape(P, -1)
        blob[:, C_WG0:C_WU0] = _pk(np.asarray(wg[e0], np.float32), KT).reshape(P, -1)
        blob[:, C_WU0:C_WD0] = _pk(np.asarray(wu[e0], np.float32), KT).reshape(P, -1)
        blob[:, C_WD0:C_WG1] = _pk(wd_pad[e0], IT).reshape(P, -1)
        blob[:, C_WG1:C_WU1] = _pk(np.asarray(wg[e1], np.float32), KT).reshape(P, -1)
        blob[:, C_WU1:C_WD1] = _pk(np.asarray(wu[e1], np.float32), KT).reshape(P, -1)
        blob[:, C_WD1:BCOLS] = _pk(wd_pad[e1], IT).reshape(P, -1)

        gmeta = np.empty((P, 144), np.float32)
        gmeta[:, 0:128] = _pk(np.ascontiguousarray(gwt[:, perm]), KT) \
            .reshape(P, -1)
        gmeta[:, 128:144] = np.tile(bias[perm], (P, 1))

        in_maps.append({"blob": blob, "hx": hx, "gmeta": gmeta})

    return in_maps


def get_nc(**kw):
    key = tuple(sorted(kw.items()))
    if key not in _BUILD_CACHE:
        _BUILD_CACHE[key] = _build(**kw)
    return _BUILD_CACHE[key]


def kernel(h, gate_w, bias, wg, wu, wd, swg, swu, swd):
    in_maps = prepare_in_maps(h, gate_w, bias, wg, wu, wd, swg, swu, swd)
    res = run_bass_kernel_spmd(get_nc(), in_maps, list(range(NCORES)))
    return np.concatenate([res.results[c]["out"] for c in range(NCORES)],
                          axis=0).astype(np.float32)


# revision 6
# speedup vs baseline: 1.0853x; 1.0853x over previous
"""DeepSeek-V2-style MoE kernel for 8 Trainium2 NeuronCores.

Strategy (expert-parallel, dense):
- 16 experts, 8 cores -> 2 experts per core. Each core computes its two
  experts' SwiGLU MLPs densely over all 1024 tokens (bf16 matmuls, fp32
  accumulate), weighted by on-device routing weights.
- The shared expert is sharded over its intermediate dim (256 of 2048 per
  core); its per-core partial seeds the routed combine, so one
  ReduceScatter(add) of the [T, H] partial produces each core's final
  128-token output shard directly.
- The gate (sigmoid + grouped top-k) runs on every core in fp32 so expert
  selection matches the fp32 reference exactly. The expert axis is permuted
  per core so each core's own experts sit at positions 0 and 1.
- All per-core tensors are packed into THREE runtime args (weight blob
  bf16, h^T fp32, gate meta fp32): the PJRT/axon exec path charges ~60us
  per input arg per execution, so arg count dominates the measured time.
  The bf16 copy of h^T is derived on device from the fp32 arg.
"""

import os
import sys

import numpy as np
import ml_dtypes

for _p in ("/opt/trn_rl_repo", os.path.expanduser("~/.axon_site/_ro/trn_rl_repo")):
    if os.path.isdir(_p) and _p not in sys.path:
        sys.path.append(_p)

import concourse.bass as bass
import concourse.mybir as mybir
import concourse.tile as tile
from concourse.bass_utils import run_bass_kernel_spmd

# problem sizes (fixed)
T, H, E, I, SI = 1024, 1024, 16, 704, 2048
P = 128
NCORES = 8
KT = H // P            # 8 contraction tiles over H
IT = 6                 # ceil(704/128) I tiles; last is 64 rows (wd zero-padded)
IPAD = IT * P          # 768
SIC = SI // NCORES     # 256: shared-expert intermediate slice per core
SICT = SIC // P        # 2
NB = 2                 # token blocks
BLK = T // NB          # 512
MSUB = BLK // P        # 4 token subtiles per block
BIG = 1.0e6
OFF = 10.0             # offset making all valid masked scores positive

# blob column layout (bf16, [P, BCOLS]); order = DMA issue order so compute
# can chase the loads: shared expert first, then e0 up, e0 down, e1 up, e1 down
C_SWG = 0
C_SWU = C_SWG + KT * SIC          # 2048
C_SWD = C_SWU + KT * SIC          # 4096
C_WG0 = C_SWD + SICT * H          # 6144
C_WU0 = C_WG0 + KT * I            # 11776
C_WD0 = C_WU0 + KT * I            # 17408
C_WG1 = C_WD0 + IT * H            # 23552
C_WU1 = C_WG1 + KT * I            # 29184
C_WD1 = C_WU1 + KT * I            # 34816
BCOLS = C_WD1 + IT * H            # 40960

F32 = mybir.dt.float32
BF16 = mybir.dt.bfloat16
ALU = mybir.AluOpType
ACTF = mybir.ActivationFunctionType

_BUILD_CACHE = {}


def _split_sync_waits(nc):
    """This walrus build allows one sync wait per instruction; move extra
    waits onto same-engine pure-wait carriers placed immediately before."""
    n_split = 0
    for f in nc.m.functions:
        for bb in f.blocks:
            out = []
            for ins in bb.instructions:
                si = ins.sync_info
                if si is not None and si.on_wait and len(si.on_wait) > 1:
                    waits = list(si.on_wait)
                    head, tail = waits[:-1], waits[-1:]
                    for i, w in enumerate(head):
                        carrier = mybir.InstEventSemaphore(
                            name=f"{ins.name}-ws{i}",
                            engine=ins.engine,
                            ins=[],
                            outs=[],
                            sync_info=mybir.SyncInfo(on_wait=[w], on_update=[]),
                        )
                        nc.register_instruction(carrier, overwrite=True)
                        out.append(carrier)
                    ins.sync_info = mybir.SyncInfo(on_wait=tail,
                                                   on_update=si.on_update)
                    n_split += 1
                out.append(ins)
            bb.instructions[:] = out
    return nc


def _build(with_collective=True, routed_reps=1, shared_reps=1, coll_reps=1):
    nc = bass.Bass(num_devices=NCORES)

    # ---- three packed runtime parameters (per-core contents host-side) ----
    blob = nc.declare_dram_parameter("blob", [P, BCOLS], BF16, isOutput=False)
    hx = nc.declare_dram_parameter("hx", [P, KT * T + 144], F32, isOutput=False)
    out = nc.declare_dram_parameter("out", [P, H], F32, isOutput=True)

    with tile.TileContext(nc) as tc:
        with (
            tc.tile_pool(name="const", bufs=1) as const,
            tc.tile_pool(name="wpool", bufs=1) as wpool,
            tc.tile_pool(name="apool", bufs=2) as apool,
            tc.tile_pool(name="stmp", bufs=2) as stmp,
            tc.tile_pool(name="part", bufs=2) as part,
            tc.tile_pool(name="rpool", bufs=1) as rpool,
            tc.tile_pool(name="pgu", bufs=4, space="PSUM") as pgu,
            tc.tile_pool(name="py", bufs=4, space="PSUM") as py,
            tc.tile_pool(name="dram", bufs=1, space="DRAM") as dram,
        ):
            # ------------- loads: big blob in 5 chunks across queues -------
            bsb = wpool.tile([P, BCOLS], BF16, name="bsb")
            nc.scalar.dma_start(out=bsb[:, C_SWG:C_WG0], in_=blob[:, C_SWG:C_WG0])
            nc.sync.dma_start(out=bsb[:, C_WG0:C_WD0], in_=blob[:, C_WG0:C_WD0])
            nc.gpsimd.dma_start(out=bsb[:, C_WD0:C_WG1], in_=blob[:, C_WD0:C_WG1])
            nc.sync.dma_start(out=bsb[:, C_WG1:C_WD1], in_=blob[:, C_WG1:C_WD1])
            nc.gpsimd.dma_start(out=bsb[:, C_WD1:BCOLS], in_=blob[:, C_WD1:BCOLS])

            swg_sb = bsb[:, C_SWG:C_SWU].rearrange("p (k c) -> p k c", k=KT)
            swu_sb = bsb[:, C_SWU:C_SWD].rearrange("p (k c) -> p k c", k=KT)
            swd_sb = bsb[:, C_SWD:C_WG0].rearrange("p (i h) -> p i h", i=SICT)
            wg_sb = [bsb[:, C_WG0:C_WU0].rearrange("p (k i) -> p k i", k=KT),
                     bsb[:, C_WG1:C_WU1].rearrange("p (k i) -> p k i", k=KT)]
            wu_sb = [bsb[:, C_WU0:C_WD0].rearrange("p (k i) -> p k i", k=KT),
                     bsb[:, C_WU1:C_WD1].rearrange("p (k i) -> p k i", k=KT)]
            wd_sb = [bsb[:, C_WD0:C_WG1].rearrange("p (i h) -> p i h", i=IT),
                     bsb[:, C_WD1:BCOLS].rearrange("p (i h) -> p i h", i=IT)]

            hx_sb = const.tile([P, KT, T], F32, name="hx_sb")
            nc.scalar.dma_start(out=hx_sb[:].rearrange("p k t -> p (k t)"),
                                in_=hx[:, 0:KT * T])
            gm_sb = const.tile([P, 144], F32, name="gm_sb")
            nc.sync.dma_start(out=gm_sb[:], in_=hx[:, KT * T:KT * T + 144])
            gw_sb = gm_sb[:, 0:128].rearrange("p (k e) -> p k e", k=KT)
            bias16 = gm_sb[:, 128:144]

            # bf16 copy of h^T for the expert matmuls
            ht_sb = const.tile([P, KT, T], BF16, name="ht_sb")
            for k in range(KT):
                nc.vector.tensor_copy(ht_sb[:, k, :], hx_sb[:, k, :])

            scores = rpool.tile([P, P], F32, name="scores")

            # ------------- shared expert (intermediate slice, all tokens) --
            As = const.tile([P, SICT, T], BF16, name="As_sh")
            ys = const.tile([P, NB * MSUB, 2, 512], BF16, name="ys")
            for rep_s in range(shared_reps):
                for si in range(SICT):
                    for b in range(NB):
                        tsl = slice(b * BLK, (b + 1) * BLK)
                        pGs = pgu.tile([P, 512], F32, name="pgs", tag="pgu")
                        pUs = pgu.tile([P, 512], F32, name="pus", tag="pgu")
                        for k in range(KT):
                            nc.tensor.matmul(
                                pGs[:, :], lhsT=swg_sb[:, k, si * P:(si + 1) * P],
                                rhs=ht_sb[:, k, tsl],
                                start=(k == 0), stop=(k == KT - 1))
                        for k in range(KT):
                            nc.tensor.matmul(
                                pUs[:, :], lhsT=swu_sb[:, k, si * P:(si + 1) * P],
                                rhs=ht_sb[:, k, tsl],
                                start=(k == 0), stop=(k == KT - 1))
                        sts = stmp.tile([P, BLK], F32, name="st", tag="st")
                        nc.scalar.activation(sts[:, :], pGs[:, :], ACTF.Silu)
                        nc.vector.tensor_tensor(As[:, si, tsl], sts[:, :],
                                                pUs[:, :], op=ALU.mult)
                if rep_s == 0:
                    for tt in range(8):
                        pg = pgu.tile([P, 512], F32, name="pgate", tag="pgu")
                        for k in range(KT):
                            nc.tensor.matmul(pg[:, :E],
                                             lhsT=hx_sb[:, k, tt * P:(tt + 1) * P],
                                             rhs=gw_sb[:, k, :],
                                             start=(k == 0), stop=(k == KT - 1))
                        nc.scalar.activation(scores[:, tt * E:(tt + 1) * E],
                                             pg[:, :E], ACTF.Sigmoid)
                for mg in range(NB * MSUB):
                    for n in range(2):
                        pYs = py.tile([P, 512], F32, name="pys", tag="py")
                        for si in range(SICT):
                            nc.tensor.matmul(
                                pYs[:, :],
                                lhsT=As[:, si, mg * P:(mg + 1) * P],
                                rhs=swd_sb[:, si, n * 512:(n + 1) * 512],
                                start=(si == 0), stop=(si == SICT - 1))
                        nc.scalar.activation(ys[:, mg, n, :], pYs[:, :],
                                             ACTF.Copy)

            # ------------- routing -------------
            sfc = rpool.tile([P, P], F32, name="sfc")
            biasb = bias16.rearrange("p (o e) -> p o e", o=1) \
                .broadcast_to([P, 8, E])
            nc.vector.tensor_tensor(
                sfc[:].rearrange("p (t e) -> p t e", t=8),
                scores[:].rearrange("p (t e) -> p t e", t=8), biasb, op=ALU.add)
            v4 = sfc[:].rearrange("p (t g e) -> p t g e", t=8, g=4, e=4)

            def t32(nm):
                return rpool.tile([P, 32], F32, name=nm)

            def v32(t):
                return t[:].rearrange("p (t g) -> p t g", t=8)

            a_, b_, c_, d_ = (v4[:, :, :, j] for j in range(4))
            m1, n1, m2, n2 = t32("m1"), t32("n1"), t32("m2"), t32("n2")
            top1, t3, t4, sec, gs = (t32(x) for x in
                                     ("top1", "t3", "t4", "sec", "gs"))
            nc.vector.tensor_tensor(v32(m1), a_, b_, op=ALU.max)
            nc.vector.tensor_tensor(v32(n1), a_, b_, op=ALU.min)
            nc.vector.tensor_tensor(v32(m2), c_, d_, op=ALU.max)
            nc.vector.tensor_tensor(v32(n2), c_, d_, op=ALU.min)
            nc.vector.tensor_tensor(top1[:], m1[:], m2[:], op=ALU.max)
            nc.vector.tensor_tensor(t3[:], m1[:], m2[:], op=ALU.min)
            nc.vector.tensor_tensor(t4[:], n1[:], n2[:], op=ALU.max)
            nc.vector.tensor_tensor(sec[:], t3[:], t4[:], op=ALU.max)
            nc.vector.tensor_tensor(gs[:], top1[:], sec[:], op=ALU.add)

            gv = gs[:].rearrange("p (t g) -> p t g", t=8)

            def t8(nm):
                return rpool.tile([P, 8], F32, name=nm)

            u1, l1, u2, l2, q1, q2, thr = (t8(x) for x in
                                           ("u1", "l1", "u2", "l2", "q1", "q2",
                                            "thr"))
            x0, x1, x2, x3 = (gv[:, :, j] for j in range(4))
            nc.vector.tensor_tensor(u1[:], x0, x1, op=ALU.max)
            nc.vector.tensor_tensor(l1[:], x0, x1, op=ALU.min)
            nc.vector.tensor_tensor(u2[:], x2, x3, op=ALU.max)
            nc.vector.tensor_tensor(l2[:], x2, x3, op=ALU.min)
            nc.vector.tensor_tensor(q1[:], u1[:], u2[:], op=ALU.min)
            nc.vector.tensor_tensor(q2[:], l1[:], l2[:], op=ALU.max)
            nc.vector.tensor_tensor(thr[:], q1[:], q2[:], op=ALU.max)

            pen = t32("pen")
            thrb = thr[:].rearrange("p (t o) -> p t o", o=1) \
                .broadcast_to([P, 8, 4])
            nc.vector.tensor_tensor(v32(pen), gv, thrb, op=ALU.is_lt)
            nc.vector.tensor_scalar_mul(pen[:], pen[:], BIG)

            masked = rpool.tile([P, P], F32, name="masked")
            mv4 = masked[:].rearrange("p (t g e) -> p t g e", t=8, g=4, e=4)
            penb = pen[:].rearrange("p (t g o) -> p t g o", t=8, o=1) \
                .broadcast_to([P, 8, 4, 4])
            nc.vector.scalar_tensor_tensor(mv4, v4, OFF, penb,
                                           op0=ALU.add, op1=ALU.subtract)

            mv3 = masked[:].rearrange("p (t e) -> p t e", t=8)
            mx = t8("mx")
            lt = rpool.tile([P, P], F32, name="lt")
            lt3 = lt[:].rearrange("p (t e) -> p t e", t=8)
            for _ in range(6):
                nc.vector.tensor_reduce(mx[:], mv3, axis=mybir.AxisListType.X,
                                        op=ALU.max)
                mxb = mx[:].rearrange("p (t o) -> p t o", o=1) \
                    .broadcast_to([P, 8, 16])
                nc.vector.tensor_tensor(lt3, mv3, mxb, op=ALU.is_lt)
                nc.vector.tensor_tensor(masked[:], lt[:], masked[:],
                                        op=ALU.mult)

            sel = rpool.tile([P, P], F32, name="sel")
            nc.vector.tensor_scalar(sel[:], masked[:], 0.0, None,
                                    op0=ALU.is_equal)
            sw = rpool.tile([P, P], F32, name="swt")
            nc.vector.tensor_tensor(sw[:], scores[:], sel[:], op=ALU.mult)
            sums = t8("sums")
            nc.vector.tensor_reduce(sums[:],
                                    sw[:].rearrange("p (t e) -> p t e", t=8),
                                    axis=mybir.AxisListType.X, op=ALU.add)
            rec = t8("rec")
            nc.vector.reciprocal(rec[:], sums[:])
            cw = [rpool.tile([P, 8], F32, name=f"cw{e}") for e in range(2)]
            swv = sw[:].rearrange("p (t e) -> p t e", t=8)
            for e in range(2):
                for tt in range(8):
                    nc.vector.scalar_tensor_tensor(
                        cw[e][:, tt:tt + 1], swv[:, tt, e:e + 1], 2.0,
                        rec[:, tt:tt + 1], op0=ALU.mult, op1=ALU.mult)

            # ------------- DRAM partials & collectives -------------
            partial = [dram.tile([T, 512], F32, name=f"partial{n}")
                       for n in range(2)]
            rs = [dram.tile([P, 512], F32, name=f"rs{n}") for n in range(2)]

            # ------------- routed experts -------------
            for rep, b in [(rep, b) for rep in range(routed_reps)
                           for b in range(NB)]:
                last_rep = rep == routed_reps - 1
                tsl = slice(b * BLK, (b + 1) * BLK)
                A = []
                for e in range(2):
                    At = apool.tile([P, IT, BLK], BF16, name=f"A{e}",
                                    tag=f"A{e}")
                    nc.vector.memset(At[P - 64:, IT - 1, :], 0.0)
                    for i in range(IT):
                        ip = P if i < IT - 1 else I - (IT - 1) * P
                        pG = pgu.tile([P, 512], F32, name="pgu", tag="pgu")
                        pU = pgu.tile([P, 512], F32, name="pgu2", tag="pgu")
                        for k in range(KT):
                            nc.tensor.matmul(
                                pG[:ip, :],
                                lhsT=wg_sb[e][:, k, i * P:i * P + ip],
                                rhs=ht_sb[:, k, tsl],
                                start=(k == 0), stop=(k == KT - 1))
                        for k in range(KT):
                            nc.tensor.matmul(
                                pU[:ip, :],
                                lhsT=wu_sb[e][:, k, i * P:i * P + ip],
                                rhs=ht_sb[:, k, tsl],
                                start=(k == 0), stop=(k == KT - 1))
                        st = stmp.tile([P, BLK], F32, name="st", tag="st")
                        nc.scalar.activation(st[:ip, :], pG[:ip, :], ACTF.Silu)
                        nc.vector.tensor_tensor(At[:ip, i, :], st[:ip, :],
                                                pU[:ip, :], op=ALU.mult)
                    A.append(At)

                for n in range(2):
                    pt = part.tile([P, MSUB, 512], F32, name="pt", tag="pt")
                    for e in range(2):
                        for m in range(MSUB):
                            pY = py.tile([P, 512], F32, name="py", tag="py")
                            for i in range(IT):
                                nc.tensor.matmul(
                                    pY[:, :],
                                    lhsT=A[e][:, i, m * P:(m + 1) * P],
                                    rhs=wd_sb[e][:, i, n * 512:(n + 1) * 512],
                                    start=(i == 0), stop=(i == IT - 1))
                            tt = b * MSUB + m
                            if e == 0:
                                # seed with the shared-expert partial
                                nc.vector.scalar_tensor_tensor(
                                    pt[:, m, :], pY[:, :], cw[0][:, tt:tt + 1],
                                    ys[:, tt, n, :], op0=ALU.mult, op1=ALU.add)
                            else:
                                nc.vector.scalar_tensor_tensor(
                                    pt[:, m, :], pY[:, :], cw[1][:, tt:tt + 1],
                                    pt[:, m, :], op0=ALU.mult, op1=ALU.add)
                    if last_rep:
                        for m in range(MSUB):
                            r0 = b * BLK + m * P
                            nc.sync.dma_start(
                                out=partial[n][r0:r0 + P, :],
                                in_=pt[:, m, :])
                    if last_rep and b == NB - 1 and with_collective:
                        for _cr in range(coll_reps):
                            nc.gpsimd.collective_compute(
                                "ReduceScatter", ALU.add,
                                replica_groups=[list(range(NCORES))],
                                ins=[partial[n][:]], outs=[rs[n][:]])

            # ------------- epilogue -------------
            for n in range(2):
                if with_collective:
                    nc.sync.dma_start(out=out[:, n * 512:(n + 1) * 512],
                                      in_=rs[n][:])
                else:
                    nc.sync.dma_start(out=out[:, n * 512:(n + 1) * 512],
                                      in_=partial[n][0:P, :])

    _split_sync_waits(nc)
    return nc


def _perm_for_core(c):
    g_sel = c >> 1
    rot = 2 * (c & 1)
    perm = [4 * g_sel + ((rot + j) % 4) for j in range(4)]
    for g in range(4):
        if g != g_sel:
            perm.extend(range(4 * g, 4 * g + 4))
    return perm


def _pk(w, k):
    """[k*P, X] -> [P, k, X] partition-major blocking."""
    return np.ascontiguousarray(
        w.reshape(k, P, w.shape[1]).transpose(1, 0, 2))


def prepare_in_maps(h, gate_w, bias, wg, wu, wd, swg, swu, swd):
    bf = ml_dtypes.bfloat16
    h = np.asarray(h, np.float32)
    gate_w = np.asarray(gate_w, np.float32)
    bias = np.asarray(bias, np.float32)

    ht32 = np.ascontiguousarray(h.T)                      # [H, T] f32
    hxm = _pk(ht32, KT).reshape(P, KT * T)                # [P, 8*1024] f32
    gwt = np.ascontiguousarray(gate_w.T)                  # [H, E] f32

    swg32 = np.asarray(swg, np.float32)
    swu32 = np.asarray(swu, np.float32)
    swd32 = np.asarray(swd, np.float32)

    wd_pad = np.zeros((E, IPAD, H), np.float32)
    wd_pad[:, :I, :] = np.asarray(wd, np.float32)

    in_maps = []
    for c in range(NCORES):
        e0, e1 = 2 * c, 2 * c + 1
        perm = _perm_for_core(c)
        csl = slice(c * SIC, (c + 1) * SIC)

        blob = np.empty((P, BCOLS), bf)
        blob[:, C_SWG:C_SWU] = _pk(swg32[:, csl], KT).reshape(P, -1)
        blob[:, C_SWU:C_SWD] = _pk(swu32[:, csl], KT).reshape(P, -1)
        blob[:, C_SWD:C_WG0] = _pk(swd32[csl, :], SICT).reshape(P, -1)
        blob[:, C_WG0:C_WU0] = _pk(np.asarray(wg[e0], np.float32), KT).reshape(P, -1)
        blob[:, C_WU0:C_WD0] = _pk(np.asarray(wu[e0], np.float32), KT).reshape(P, -1)
        blob[:, C_WD0:C_WG1] = _pk(wd_pad[e0], IT).reshape(P, -1)
        blob[:, C_WG1:C_WU1] = _pk(np.asarray(wg[e1], np.float32), KT).reshape(P, -1)
        blob[:, C_WU1:C_WD1] = _pk(np.asarray(wu[e1], np.float32), KT).reshape(P, -1)
        blob[:, C_WD1:BCOLS] = _pk(wd_pad[e1], IT).reshape(P, -1)

        hx = np.empty((P, KT * T + 144), np.float32)
        hx[:, 0:KT * T] = hxm
        hx[:, KT * T:KT * T + 128] = _pk(
            np.ascontiguousarray(gwt[:, perm]), KT).reshape(P, -1)
        hx[:, KT * T + 128:] = np.tile(bias[perm], (P, 1))

        in_maps.append({"blob": blob, "hx": hx})

    return in_maps


def get_nc(**kw):
    key = tuple(sorted(kw.items()))
    if key not in _BUILD_CACHE:
        _BUILD_CACHE[key] = _build(**kw)
    return _BUILD_CACHE[key]


def kernel(h, gate_w, bias, wg, wu, wd, swg, swu, swd):
    in_maps = prepare_in_maps(h, gate_w, bias, wg, wu, wd, swg, swu, swd)
    res = run_bass_kernel_spmd(get_nc(), in_maps, list(range(NCORES)))
    return np.concatenate([res.results[c]["out"] for c in range(NCORES)],
                          axis=0).astype(np.float32)


# revision 12
# speedup vs baseline: 2.7226x; 2.5086x over previous
"""DeepSeek-V2-style MoE kernel for 8 Trainium2 NeuronCores.

Strategy (expert-parallel, dense):
- 16 experts, 8 cores -> 2 experts per core. Each core computes its two
  experts' SwiGLU MLPs densely over all 1024 tokens (bf16 matmuls, fp32
  accumulate), weighted by on-device routing weights.
- The shared expert is sharded over its intermediate dim (256 of 2048 per
  core); its per-core partial seeds the routed combine, so one
  ReduceScatter(add) of the [T, H] partial produces each core's final
  128-token output shard directly.
- The gate (sigmoid + grouped top-k) runs on every core in fp32 so expert
  selection matches the fp32 reference exactly. The expert axis is permuted
  per core so each core's own experts sit at positions 0 and 1.
- All per-core tensors are packed into THREE runtime args (weight blob
  bf16, h^T fp32, gate meta fp32): the PJRT/axon exec path charges ~60us
  per input arg per execution, so arg count dominates the measured time.
  The bf16 copy of h^T is derived on device from the fp32 arg.
"""

import os
import sys

import numpy as np
import ml_dtypes

for _p in ("/opt/trn_rl_repo", os.path.expanduser("~/.axon_site/_ro/trn_rl_repo")):
    if os.path.isdir(_p) and _p not in sys.path:
        sys.path.append(_p)

import concourse.bass as bass
import concourse.mybir as mybir
import concourse.tile as tile
from concourse.bass_utils import run_bass_kernel_spmd

# problem sizes (fixed)
T, H, E, I, SI = 1024, 1024, 16, 704, 2048
P = 128
NCORES = 8
KT = H // P            # 8 contraction tiles over H
IT = 6                 # ceil(704/128) I tiles; last is 64 rows (wd zero-padded)
IPAD = IT * P          # 768
SIC = SI // NCORES     # 256: shared-expert intermediate slice per core
SICT = SIC // P        # 2
NB = 2                 # token blocks
BLK = T // NB          # 512
MSUB = BLK // P        # 4 token subtiles per block
BIG = 1.0e6
OFF = 10.0             # offset making all valid masked scores positive

# blob column layout (bf16, [P, BCOLS]); order = DMA issue order so compute
# can chase the loads: shared expert first, then e0 up, e0 down, e1 up, e1 down
C_SWG = 0
C_SWU = C_SWG + KT * SIC          # 2048
C_SWD = C_SWU + KT * SIC          # 4096
C_WG0 = C_SWD + SICT * H          # 6144
C_WU0 = C_WG0 + KT * I            # 11776
C_WD0 = C_WU0 + KT * I            # 17408
C_WG1 = C_WD0 + IT * H            # 23552
C_WU1 = C_WG1 + KT * I            # 29184
C_WD1 = C_WU1 + KT * I            # 34816
BCOLS = C_WD1 + IT * H            # 40960

F32 = mybir.dt.float32
BF16 = mybir.dt.bfloat16
ALU = mybir.AluOpType
ACTF = mybir.ActivationFunctionType

_BUILD_CACHE = {}


def _split_sync_waits(nc):
    """This walrus build allows one sync wait per instruction; move extra
    waits onto same-engine pure-wait carriers placed immediately before."""
    n_split = 0
    for f in nc.m.functions:
        for bb in f.blocks:
            out = []
            for ins in bb.instructions:
                si = ins.sync_info
                if si is not None and si.on_wait and len(si.on_wait) > 1:
                    waits = list(si.on_wait)
                    head, tail = waits[:-1], waits[-1:]
                    for i, w in enumerate(head):
                        carrier = mybir.InstEventSemaphore(
                            name=f"{ins.name}-ws{i}",
                            engine=ins.engine,
                            ins=[],
                            outs=[],
                            sync_info=mybir.SyncInfo(on_wait=[w], on_update=[]),
                        )
                        nc.register_instruction(carrier, overwrite=True)
                        out.append(carrier)
                    ins.sync_info = mybir.SyncInfo(on_wait=tail,
                                                   on_update=si.on_update)
                    n_split += 1
                out.append(ins)
            bb.instructions[:] = out
    return nc


def _build(with_collective=True, routed_reps=1, shared_reps=1, coll_reps=1):
    nc = bass.Bass(num_devices=NCORES)

    # ---- three packed runtime parameters (per-core contents host-side) ----
    blob = nc.declare_dram_parameter("blob", [P, BCOLS], BF16, isOutput=False)
    hx = nc.declare_dram_parameter("hx", [P, KT * T + 144], F32, isOutput=False)
    out = nc.declare_dram_parameter("out", [P, H], F32, isOutput=True)

    with tile.TileContext(nc) as tc:
        with (
            tc.tile_pool(name="const", bufs=1) as const,
            tc.tile_pool(name="wpool", bufs=1) as wpool,
            tc.tile_pool(name="apool", bufs=2) as apool,
            tc.tile_pool(name="stmp", bufs=2) as stmp,
            tc.tile_pool(name="part", bufs=2) as part,
            tc.tile_pool(name="rpool", bufs=1) as rpool,
            tc.tile_pool(name="pgu", bufs=4, space="PSUM") as pgu,
            tc.tile_pool(name="py", bufs=4, space="PSUM") as py,
            tc.tile_pool(name="dram", bufs=1, space="DRAM") as dram,
        ):
            # ------------- loads: hx per-k chunks first, blob chunks chase --
            hx_sb = const.tile([P, KT, T], F32, name="hx_sb")
            ht_sb = const.tile([P, KT, T], BF16, name="ht_sb")
            _heng = [nc.scalar, nc.gpsimd, nc.sync]
            for k in range(KT):
                _heng[k % 3].dma_start(out=hx_sb[:, k, :],
                                       in_=hx[:, k * T:(k + 1) * T])
                nc.vector.tensor_copy(ht_sb[:, k, :], hx_sb[:, k, :])

            bsb = wpool.tile([P, BCOLS], BF16, name="bsb")
            nc.scalar.dma_start(out=bsb[:, C_SWG:C_WG0], in_=blob[:, C_SWG:C_WG0])
            nc.sync.dma_start(out=bsb[:, C_WG0:C_WD0], in_=blob[:, C_WG0:C_WD0])
            nc.gpsimd.dma_start(out=bsb[:, C_WD0:C_WG1], in_=blob[:, C_WD0:C_WG1])
            nc.sync.dma_start(out=bsb[:, C_WG1:C_WD1], in_=blob[:, C_WG1:C_WD1])
            nc.gpsimd.dma_start(out=bsb[:, C_WD1:BCOLS], in_=blob[:, C_WD1:BCOLS])

            swg_sb = bsb[:, C_SWG:C_SWU].rearrange("p (k c) -> p k c", k=KT)
            swu_sb = bsb[:, C_SWU:C_SWD].rearrange("p (k c) -> p k c", k=KT)
            swd_sb = bsb[:, C_SWD:C_WG0].rearrange("p (i h) -> p i h", i=SICT)
            wg_sb = [bsb[:, C_WG0:C_WU0].rearrange("p (k i) -> p k i", k=KT),
                     bsb[:, C_WG1:C_WU1].rearrange("p (k i) -> p k i", k=KT)]
            wu_sb = [bsb[:, C_WU0:C_WD0].rearrange("p (k i) -> p k i", k=KT),
                     bsb[:, C_WU1:C_WD1].rearrange("p (k i) -> p k i", k=KT)]
            wd_sb = [bsb[:, C_WD0:C_WG1].rearrange("p (i h) -> p i h", i=IT),
                     bsb[:, C_WD1:BCOLS].rearrange("p (i h) -> p i h", i=IT)]

            gm_sb = const.tile([P, 144], F32, name="gm_sb")
            nc.sync.dma_start(out=gm_sb[:], in_=hx[:, KT * T:KT * T + 144])
            gw_sb = gm_sb[:, 0:128].rearrange("p (k e) -> p k e", k=KT)
            bias16 = gm_sb[:, 128:144]

            scores = rpool.tile([P, P], F32, name="scores")

            # ------------- shared expert (intermediate slice, all tokens) --
            As = const.tile([P, SICT, T], BF16, name="As_sh")
            ys = const.tile([P, NB * MSUB, 2, 512], BF16, name="ys")
            for rep_s in range(shared_reps):
                for si in range(SICT):
                    for b in range(NB):
                        tsl = slice(b * BLK, (b + 1) * BLK)
                        pGs = pgu.tile([P, 512], F32, name="pgs", tag="pgu")
                        pUs = pgu.tile([P, 512], F32, name="pus", tag="pgu")
                        for k in range(KT):
                            nc.tensor.matmul(
                                pGs[:, :], lhsT=swg_sb[:, k, si * P:(si + 1) * P],
                                rhs=ht_sb[:, k, tsl],
                                start=(k == 0), stop=(k == KT - 1))
                        for k in range(KT):
                            nc.tensor.matmul(
                                pUs[:, :], lhsT=swu_sb[:, k, si * P:(si + 1) * P],
                                rhs=ht_sb[:, k, tsl],
                                start=(k == 0), stop=(k == KT - 1))
                        sts = stmp.tile([P, BLK], F32, name="st", tag="st")
                        nc.scalar.activation(sts[:, :], pGs[:, :], ACTF.Silu)
                        nc.vector.tensor_tensor(As[:, si, tsl], sts[:, :],
                                                pUs[:, :], op=ALU.mult)
                if rep_s == 0:
                    for tt in range(8):
                        pg = pgu.tile([P, 512], F32, name="pgate", tag="pgu")
                        for k in range(KT):
                            nc.tensor.matmul(pg[:, :E],
                                             lhsT=hx_sb[:, k, tt * P:(tt + 1) * P],
                                             rhs=gw_sb[:, k, :],
                                             start=(k == 0), stop=(k == KT - 1))
                        nc.scalar.activation(scores[:, tt * E:(tt + 1) * E],
                                             pg[:, :E], ACTF.Sigmoid)
                for mg in range(NB * MSUB):
                    for n in range(2):
                        pYs = py.tile([P, 512], F32, name="pys", tag="py")
                        for si in range(SICT):
                            nc.tensor.matmul(
                                pYs[:, :],
                                lhsT=As[:, si, mg * P:(mg + 1) * P],
                                rhs=swd_sb[:, si, n * 512:(n + 1) * 512],
                                start=(si == 0), stop=(si == SICT - 1))
                        nc.scalar.activation(ys[:, mg, n, :], pYs[:, :],
                                             ACTF.Copy)

            # ------------- routing -------------
            sfc = rpool.tile([P, P], F32, name="sfc")
            biasb = bias16.rearrange("p (o e) -> p o e", o=1) \
                .broadcast_to([P, 8, E])
            nc.vector.tensor_tensor(
                sfc[:].rearrange("p (t e) -> p t e", t=8),
                scores[:].rearrange("p (t e) -> p t e", t=8), biasb, op=ALU.add)
            v4 = sfc[:].rearrange("p (t g e) -> p t g e", t=8, g=4, e=4)

            def t32(nm):
                return rpool.tile([P, 32], F32, name=nm)

            def v32(t):
                return t[:].rearrange("p (t g) -> p t g", t=8)

            a_, b_, c_, d_ = (v4[:, :, :, j] for j in range(4))
            m1, n1, m2, n2 = t32("m1"), t32("n1"), t32("m2"), t32("n2")
            top1, t3, t4, sec, gs = (t32(x) for x in
                                     ("top1", "t3", "t4", "sec", "gs"))
            nc.vector.tensor_tensor(v32(m1), a_, b_, op=ALU.max)
            nc.vector.tensor_tensor(v32(n1), a_, b_, op=ALU.min)
            nc.vector.tensor_tensor(v32(m2), c_, d_, op=ALU.max)
            nc.vector.tensor_tensor(v32(n2), c_, d_, op=ALU.min)
            nc.vector.tensor_tensor(top1[:], m1[:], m2[:], op=ALU.max)
            nc.vector.tensor_tensor(t3[:], m1[:], m2[:], op=ALU.min)
            nc.vector.tensor_tensor(t4[:], n1[:], n2[:], op=ALU.max)
            nc.vector.tensor_tensor(sec[:], t3[:], t4[:], op=ALU.max)
            nc.vector.tensor_tensor(gs[:], top1[:], sec[:], op=ALU.add)

            gv = gs[:].rearrange("p (t g) -> p t g", t=8)

            def t8(nm):
                return rpool.tile([P, 8], F32, name=nm)

            u1, l1, u2, l2, q1, q2, thr = (t8(x) for x in
                                           ("u1", "l1", "u2", "l2", "q1", "q2",
                                            "thr"))
            x0, x1, x2, x3 = (gv[:, :, j] for j in range(4))
            nc.vector.tensor_tensor(u1[:], x0, x1, op=ALU.max)
            nc.vector.tensor_tensor(l1[:], x0, x1, op=ALU.min)
            nc.vector.tensor_tensor(u2[:], x2, x3, op=ALU.max)
            nc.vector.tensor_tensor(l2[:], x2, x3, op=ALU.min)
            nc.vector.tensor_tensor(q1[:], u1[:], u2[:], op=ALU.min)
            nc.vector.tensor_tensor(q2[:], l1[:], l2[:], op=ALU.max)
            nc.vector.tensor_tensor(thr[:], q1[:], q2[:], op=ALU.max)

            pen = t32("pen")
            thrb = thr[:].rearrange("p (t o) -> p t o", o=1) \
                .broadcast_to([P, 8, 4])
            nc.vector.tensor_tensor(v32(pen), gv, thrb, op=ALU.is_lt)
            nc.vector.tensor_scalar_mul(pen[:], pen[:], BIG)

            masked = rpool.tile([P, P], F32, name="masked")
            mv4 = masked[:].rearrange("p (t g e) -> p t g e", t=8, g=4, e=4)
            penb = pen[:].rearrange("p (t g o) -> p t g o", t=8, o=1) \
                .broadcast_to([P, 8, 4, 4])
            nc.vector.scalar_tensor_tensor(mv4, v4, OFF, penb,
                                           op0=ALU.add, op1=ALU.subtract)

            mv3 = masked[:].rearrange("p (t e) -> p t e", t=8)
            mx = t8("mx")
            lt = rpool.tile([P, P], F32, name="lt")
            lt3 = lt[:].rearrange("p (t e) -> p t e", t=8)
            for _ in range(6):
                nc.vector.tensor_reduce(mx[:], mv3, axis=mybir.AxisListType.X,
                                        op=ALU.max)
                mxb = mx[:].rearrange("p (t o) -> p t o", o=1) \
                    .broadcast_to([P, 8, 16])
                nc.vector.tensor_tensor(lt3, mv3, mxb, op=ALU.is_lt)
                nc.vector.tensor_tensor(masked[:], lt[:], masked[:],
                                        op=ALU.mult)

            sel = rpool.tile([P, P], F32, name="sel")
            nc.vector.tensor_scalar(sel[:], masked[:], 0.0, None,
                                    op0=ALU.is_equal)
            sw = rpool.tile([P, P], F32, name="swt")
            nc.vector.tensor_tensor(sw[:], scores[:], sel[:], op=ALU.mult)
            sums = t8("sums")
            nc.vector.tensor_reduce(sums[:],
                                    sw[:].rearrange("p (t e) -> p t e", t=8),
                                    axis=mybir.AxisListType.X, op=ALU.add)
            rec = t8("rec")
            nc.vector.reciprocal(rec[:], sums[:])
            cw = [rpool.tile([P, 8], F32, name=f"cw{e}") for e in range(2)]
            swv = sw[:].rearrange("p (t e) -> p t e", t=8)
            for e in range(2):
                for tt in range(8):
                    nc.vector.scalar_tensor_tensor(
                        cw[e][:, tt:tt + 1], swv[:, tt, e:e + 1], 2.0,
                        rec[:, tt:tt + 1], op0=ALU.mult, op1=ALU.mult)

            # ------------- DRAM partials & collectives -------------
            partial = [dram.tile([T, 512], BF16, name=f"partial{n}")
                       for n in range(2)]
            rs = [dram.tile([P, 512], BF16, name=f"rs{n}") for n in range(2)]

            # ------------- routed experts -------------
            for rep, b in [(rep, b) for rep in range(routed_reps)
                           for b in range(NB)]:
                last_rep = rep == routed_reps - 1
                tsl = slice(b * BLK, (b + 1) * BLK)
                A = []
                for e in range(2):
                    At = apool.tile([P, IT, BLK], BF16, name=f"A{e}",
                                    tag=f"A{e}")
                    nc.vector.memset(At[P - 64:, IT - 1, :], 0.0)
                    for i in range(IT):
                        ip = P if i < IT - 1 else I - (IT - 1) * P
                        pG = pgu.tile([P, 512], F32, name="pgu", tag="pgu")
                        pU = pgu.tile([P, 512], F32, name="pgu2", tag="pgu")
                        for k in range(KT):
                            nc.tensor.matmul(
                                pG[:ip, :],
                                lhsT=wg_sb[e][:, k, i * P:i * P + ip],
                                rhs=ht_sb[:, k, tsl],
                                start=(k == 0), stop=(k == KT - 1))
                        for k in range(KT):
                            nc.tensor.matmul(
                                pU[:ip, :],
                                lhsT=wu_sb[e][:, k, i * P:i * P + ip],
                                rhs=ht_sb[:, k, tsl],
                                start=(k == 0), stop=(k == KT - 1))
                        st = stmp.tile([P, BLK], F32, name="st", tag="st")
                        nc.scalar.activation(st[:ip, :], pG[:ip, :], ACTF.Silu)
                        nc.vector.tensor_tensor(At[:ip, i, :], st[:ip, :],
                                                pU[:ip, :], op=ALU.mult)
                    A.append(At)

                for n in range(2):
                    pt = part.tile([P, MSUB, 512], BF16, name="pt", tag="pt")
                    for e in range(2):
                        for m in range(MSUB):
                            pY = py.tile([P, 512], F32, name="py", tag="py")
                            for i in range(IT):
                                nc.tensor.matmul(
                                    pY[:, :],
                                    lhsT=A[e][:, i, m * P:(m + 1) * P],
                                    rhs=wd_sb[e][:, i, n * 512:(n + 1) * 512],
                                    start=(i == 0), stop=(i == IT - 1))
                            tt = b * MSUB + m
                            if e == 0:
                                # seed with the shared-expert partial
                                nc.vector.scalar_tensor_tensor(
                                    pt[:, m, :], pY[:, :], cw[0][:, tt:tt + 1],
                                    ys[:, tt, n, :], op0=ALU.mult, op1=ALU.add)
                            else:
                                nc.vector.scalar_tensor_tensor(
                                    pt[:, m, :], pY[:, :], cw[1][:, tt:tt + 1],
                                    pt[:, m, :], op0=ALU.mult, op1=ALU.add)
                    if last_rep:
                        for m in range(MSUB):
                            r0 = b * BLK + m * P
                            nc.sync.dma_start(
                                out=partial[n][r0:r0 + P, :],
                                in_=pt[:, m, :])
                    if last_rep and b == NB - 1 and with_collective:
                        for _cr in range(coll_reps):
                            nc.gpsimd.collective_compute(
                                "ReduceScatter", ALU.add,
                                replica_groups=[list(range(NCORES))],
                                ins=[partial[n][:]], outs=[rs[n][:]])

            # ------------- epilogue (bf16 -> f32 conversion) -------------
            for n in range(2):
                rs_sb = stmp.tile([P, 512], BF16, name=f"rs_sb{n}")
                o_sb = stmp.tile([P, 512], F32, name=f"o_sb{n}")
                src = rs[n][:] if with_collective else partial[n][0:P, :]
                nc.sync.dma_start(out=rs_sb[:], in_=src)
                nc.scalar.activation(o_sb[:], rs_sb[:], ACTF.Copy)
                nc.sync.dma_start(out=out[:, n * 512:(n + 1) * 512],
                                  in_=o_sb[:])

    _split_sync_waits(nc)
    return nc


def _perm_for_core(c):
    g_sel = c >> 1
    rot = 2 * (c & 1)
    perm = [4 * g_sel + ((rot + j) % 4) for j in range(4)]
    for g in range(4):
        if g != g_sel:
            perm.extend(range(4 * g, 4 * g + 4))
    return perm


def _pk(w, k):
    """[k*P, X] -> [P, k, X] partition-major blocking."""
    return np.ascontiguousarray(
        w.reshape(k, P, w.shape[1]).transpose(1, 0, 2))


def prepare_in_maps(h, gate_w, bias, wg, wu, wd, swg, swu, swd):
    bf = ml_dtypes.bfloat16
    h = np.asarray(h, np.float32)
    gate_w = np.asarray(gate_w, np.float32)
    bias = np.asarray(bias, np.float32)

    ht32 = np.ascontiguousarray(h.T)                      # [H, T] f32
    hxm = _pk(ht32, KT).reshape(P, KT * T)                # [P, 8*1024] f32
    gwt = np.ascontiguousarray(gate_w.T)                  # [H, E] f32

    swg32 = np.asarray(swg, np.float32)
    swu32 = np.asarray(swu, np.float32)
    swd32 = np.asarray(swd, np.float32)

    wd_pad = np.zeros((E, IPAD, H), np.float32)
    wd_pad[:, :I, :] = np.asarray(wd, np.float32)

    in_maps = []
    for c in range(NCORES):
        e0, e1 = 2 * c, 2 * c + 1
        perm = _perm_for_core(c)
        csl = slice(c * SIC, (c + 1) * SIC)

        blob = np.empty((P, BCOLS), bf)
        blob[:, C_SWG:C_SWU] = _pk(swg32[:, csl], KT).reshape(P, -1)
        blob[:, C_SWU:C_SWD] = _pk(swu32[:, csl], KT).reshape(P, -1)
        blob[:, C_SWD:C_WG0] = _pk(swd32[csl, :], SICT).reshape(P, -1)
        blob[:, C_WG0:C_WU0] = _pk(np.asarray(wg[e0], np.float32), KT).reshape(P, -1)
        blob[:, C_WU0:C_WD0] = _pk(np.asarray(wu[e0], np.float32), KT).reshape(P, -1)
        blob[:, C_WD0:C_WG1] = _pk(wd_pad[e0], IT).reshape(P, -1)
        blob[:, C_WG1:C_WU1] = _pk(np.asarray(wg[e1], np.float32), KT).reshape(P, -1)
        blob[:, C_WU1:C_WD1] = _pk(np.asarray(wu[e1], np.float32), KT).reshape(P, -1)
        blob[:, C_WD1:BCOLS] = _pk(wd_pad[e1], IT).reshape(P, -1)

        hx = np.empty((P, KT * T + 144), np.float32)
        hx[:, 0:KT * T] = hxm
        hx[:, KT * T:KT * T + 128] = _pk(
            np.ascontiguousarray(gwt[:, perm]), KT).reshape(P, -1)
        hx[:, KT * T + 128:] = np.tile(bias[perm], (P, 1))

        in_maps.append({"blob": blob, "hx": hx})

    return in_maps


def get_nc(**kw):
    key = tuple(sorted(kw.items()))
    if key not in _BUILD_CACHE:
        _BUILD_CACHE[key] = _build(**kw)
    return _BUILD_CACHE[key]


def kernel(h, gate_w, bias, wg, wu, wd, swg, swu, swd):
    in_maps = prepare_in_maps(h, gate_w, bias, wg, wu, wd, swg, swu, swd)
    res = run_bass_kernel_spmd(get_nc(), in_maps, list(range(NCORES)))
    return np.concatenate([res.results[c]["out"] for c in range(NCORES)],
                          axis=0).astype(np.float32)
